# revision 28
# baseline (speedup 1.0000x reference)
"""Multi-head self-attention (B=2, C=512, H=W=64, 8 heads) on 8 Trainium2 cores.

Sharding: data-parallel over B x head-parallel (2 heads/core). Core c handles
batch b = c//4 and heads {2*(c%4), 2*(c%4)+1} -- a contiguous 128-wide slice of
the 512-dim channel space.

Everything is laid out to avoid transposes entirely:
  - x[b] viewed as [C, S] is tok^T already (S = H*W = 4096 tokens).
  - q^T, k^T computed as [d2=128, S] (both heads stacked on partitions).
  - scores are computed TRANSPOSED: scoresT[t, s] = sum_d kT[d,t] qT[d,s],
    so softmax's exp is along the free dim and attn.V contracts t on partitions.
  - No max-subtraction needed: scores/8 ~ N(0, 0.33), exp never overflows.
  - The softmax denominator is obtained by appending a ones-column to V:
    one matmul yields both attn.V and sum(exp) rows.
  - Normalization (1/denom, varies along free dim) commutes with nothing on
    the partition axis, so it is done with a GPSIMD partition-broadcast plus
    a DVE multiply.
  - Output projection is input-column sharded: each core contributes
    attn_out[:, d_slice] @ Wp[:, d_slice].T; host sums the 4 partials per b.
    The V bias contribution (bv_slice @ WpT_slice) is folded into a
    host-precomputed per-core projection bias, so V needs no on-device bias.

All matmuls run as float32r (single-pass reduced-precision fp32, ~1.5e-4 max
rel err, ~3x faster than 2-pass fp32). exp runs on the scalar (ACT) engine
(33.5M exps/core ~ 276us busy); the PE stream (scores + attn.V, ~1.15us per
128-key x 1024-query unit) is software-pipelined against it: scores(t+1) is
issued before attn.V(t) so the PE never stalls on exp and the HAM clock
throttle stays disengaged (cold-clock matmuls are ~2x slower).

Measured on 8 axon-attached trn2 cores: ~390us HW exec, overall rel err
~6.8e-5 vs the fp32 jax reference (error entirely from f32r rounding).
"""

import os
import sys

sys.path.insert(0, "/opt/trn_rl_repo")

import numpy as np

NCORES = 8
B, C, HH, WW = 2, 512, 64, 64
S = HH * WW            # 4096 tokens
NH, D = 8, 64          # heads, head dim
DSL = 128              # per-core d-slice (2 heads)
CC = C // 128          # 4 contraction chunks
TCH = S // 128         # 32 key chunks
SBLK = 1024            # queries per attention block
NSB = S // SBLK        # 4
NSC = S // 512         # 8 (512-wide matmul slices)

_cached = {}

LAST_EXEC_NS = None
LAST_RESULTS = None


def _build():
    import concourse.mybir as mybir
    import concourse.tile as tile
    from bass_rust import add_dep_helper
    from concourse import bacc

    f32 = mybir.dt.float32
    f32r = mybir.dt.float32r
    AF = mybir.ActivationFunctionType

    nc = bacc.Bacc("TRN2", target_bir_lowering=False, debug=False,
                   num_devices=NCORES)

    xb = nc.dram_tensor("xb", [C, S], f32r, kind="ExternalInput")
    wq = nc.dram_tensor("wq", [128, CC, 128], f32r, kind="ExternalInput")
    wk = nc.dram_tensor("wk", [128, CC, 128], f32r, kind="ExternalInput")
    wv = nc.dram_tensor("wv", [128, CC, 128], f32r, kind="ExternalInput")
    wp = nc.dram_tensor("wp", [128, CC, 128], f32r, kind="ExternalInput")
    bqk = nc.dram_tensor("bqk", [128, 2], f32, kind="ExternalInput")
    pbias = nc.dram_tensor("pbias", [128, CC], f32, kind="ExternalInput")
    o = nc.dram_tensor("o", [C, S], f32, kind="ExternalOutput")

    with tile.TileContext(nc) as tc:
        with (
            tc.tile_pool(name="weights", bufs=1) as wpool,
            tc.tile_pool(name="tok", bufs=1) as tokpool,
            tc.tile_pool(name="qkv", bufs=1) as qkvpool,
            tc.tile_pool(name="exps", bufs=4) as exppool,
            tc.tile_pool(name="norm", bufs=4) as normpool,
            tc.tile_pool(name="outp", bufs=3) as outpool,
        ):
            wq_sb = wpool.tile([128, CC, 128], f32r, name="wq_sb")
            nc.sync.dma_start(out=wq_sb[:], in_=wq.ap())
            wk_sb = wpool.tile([128, CC, 128], f32r, name="wk_sb")
            nc.sync.dma_start(out=wk_sb[:], in_=wk.ap())
            wv_sb = wpool.tile([128, CC, 128], f32r, name="wv_sb")
            nc.sync.dma_start(out=wv_sb[:], in_=wv.ap())
            wp_sb = wpool.tile([128, CC, 128], f32r, name="wp_sb")
            nc.sync.dma_start(out=wp_sb[:], in_=wp.ap())
            bqk_sb = wpool.tile([128, 2], f32, name="bqk_sb")
            nc.sync.dma_start(out=bqk_sb[:], in_=bqk.ap())
            pb_sb = wpool.tile([128, CC], f32, name="pb_sb")
            nc.sync.dma_start(out=pb_sb[:], in_=pbias.ap())

            # tok^T in [partition, c_chunk, s] layout; DMA rearranges rows.
            tok_sb = tokpool.tile([128, CC, S], f32r, name="tok_sb")
            x_re = xb.ap().rearrange("(cc p) s -> p cc s", p=128)
            for qtr in range(4):
                for cc in range(CC):
                    for hf in range(2):
                        a = qtr * (S // 4) + hf * (S // 8)
                        sl = slice(a, a + S // 8)
                        nc.sync.dma_start(out=tok_sb[:, cc, sl],
                                          in_=x_re[:, cc, sl])

            qT2 = qkvpool.tile([128, S], f32r, name="qT2")
            # k^T is stored twice, zero-padded to a full K=128 contraction:
            # kTp0 = [kT_pair0; 0], kTp1 = [0; kT_pair1]. A K=128 f32r matmul
            # streams 2x faster than K=64 (measured 336 vs 526 ns), and the
            # zero rows nullify the other pair's rows of the shared qT2.
            kTp0 = qkvpool.tile([128, S], f32r, name="kTp0")
            kTp1 = qkvpool.tile([128, S], f32r, name="kTp1")
            zer32 = qkvpool.tile([64, 512], f32, name="zer32")
            nc.vector.memset(zer32[:], 0.0)
            nc.vector.tensor_copy(kTp0[64:128, 0:512], zer32[:])
            for j in range(1, 8):
                nc.vector.tensor_copy(kTp0[64:128, j * 512:(j + 1) * 512], zer32[:])
            for j in range(8):
                nc.vector.tensor_copy(kTp1[0:64, j * 512:(j + 1) * 512], zer32[:])
            # v with a ones column per key-chunk, per pair: [t, chunk, 65]
            v1_0 = qkvpool.tile([128, TCH, 65], f32r, name="v1_0")
            v1_1 = qkvpool.tile([128, TCH, 65], f32r, name="v1_1")
            ones32 = qkvpool.tile([128, TCH], f32, name="ones32")
            nc.vector.memset(ones32[:], 1.0)
            nc.vector.tensor_copy(v1_0[:, :, 64], ones32[:])
            nc.vector.tensor_copy(v1_1[:, :, 64], ones32[:])

            outT2 = qkvpool.tile([128, S], f32r, name="outT2")

            # ---- fused Q/K/V prologue, quarter-major so compute chases
            # the x DMA. V is computed transposed (efficient N=512 matmuls)
            # and flipped into [t, d] layout with PE transposes.
            ident = qkvpool.tile([128, 128], f32, name="ident")
            from concourse.masks import make_identity
            make_identity(nc, ident[:])
            ctx_psav = tc.tile_pool(name="psav", bufs=1, space="PSUM")
            pavp = ctx_psav.__enter__()
            psavs = {}
            exp_state = {"emitted": 0, "prev": None, "early": []}
            with (
                tc.tile_pool(name="psqk", bufs=2, space="PSUM") as pqkp,
                tc.tile_pool(name="pst", bufs=2, space="PSUM") as pstp,
                tc.tile_pool(name="pssce", bufs=1, space="PSUM") as pscep,
                tc.tile_pool(name="vt", bufs=2) as vtpool,
            ):
                units = [(sb, pair, tch)
                         for sb in range(NSB) for pair in range(2)
                         for tch in range(TCH)]

                def early_scores(u):
                    sb, pair, tch = u
                    kTp = kTp0 if pair == 0 else kTp1
                    s0, t0 = sb * SBLK, tch * 128
                    pssc = pscep.tile([128, SBLK], f32, name="pssce")
                    for nn in range(SBLK // 512):
                        nc.tensor.matmul(
                            pssc[:, nn * 512:(nn + 1) * 512],
                            kTp[:, t0:t0 + 128],
                            qT2[:, s0 + nn * 512:s0 + (nn + 1) * 512],
                            start=True, stop=True,
                        )
                    expT = exppool.tile([128, SBLK], f32r, name="expT")
                    nc.scalar.activation(expT[:], pssc[:], AF.Exp, scale=0.125)
                    return expT

                def early_av(u, expT):
                    sb, pair, tch = u
                    v1 = v1_0 if pair == 0 else v1_1
                    if tch == 0:
                        psavs[(sb, pair)] = pavp.tile([65, SBLK], f32,
                                                      name="psav")
                    psav = psavs[(sb, pair)]
                    for nn in range(SBLK // 512):
                        nc.tensor.matmul(
                            psav[:, nn * 512:(nn + 1) * 512],
                            v1[:, tch, :],
                            expT[:, nn * 512:(nn + 1) * 512],
                            start=(tch == 0), stop=(tch == TCH - 1),
                        )

                def early_advance(k):
                    st = exp_state
                    while st["emitted"] < k:
                        i = st["emitted"]
                        cur = early_scores(units[i])
                        if i > 0:
                            early_av(units[i - 1], st["prev"])
                        st["prev"] = cur
                        st["emitted"] = i + 1

                for qtr in range(4):
                    for which in range(3):
                        w_sb = (wq_sb, wk_sb, wv_sb)[which]
                        for scq in range(2):
                            sc = qtr * 2 + scq
                            s0 = sc * 512
                            psqk = pqkp.tile([128, 512], f32, name="psqk")
                            for cc in range(CC):
                                nc.tensor.matmul(
                                    psqk[:],
                                    w_sb[:, cc, :],
                                    tok_sb[:, cc, s0:s0 + 512],
                                    start=(cc == 0), stop=(cc == CC - 1),
                                )
                            if which == 0:
                                nc.vector.tensor_scalar_add(
                                    qT2[:, s0:s0 + 512], psqk[:], bqk_sb[:, 0:1]
                                )
                            elif which == 1:
                                nc.vector.tensor_scalar_add(
                                    kTp0[0:64, s0:s0 + 512], psqk[0:64, :],
                                    bqk_sb[0:64, 1:2]
                                )
                                nc.vector.tensor_scalar_add(
                                    kTp1[64:128, s0:s0 + 512], psqk[64:128, :],
                                    bqk_sb[64:128, 1:2]
                                )
                            else:
                                vt = vtpool.tile([128, 512], f32r, name="vt")
                                nc.vector.tensor_copy(vt[:], psqk[:])
                                for tt in range(4):
                                    tch = sc * 4 + tt
                                    pst = pstp.tile([128, 128], f32, name="pst")
                                    nc.tensor.transpose(
                                        pst[:],
                                        vt[:, tt * 128:(tt + 1) * 128].bitcast(f32),
                                        ident[:],
                                    )
                                    nc.vector.tensor_copy(
                                        v1_0[:, tch, 0:64], pst[:, 0:64]
                                    )
                                    nc.vector.tensor_copy(
                                        v1_1[:, tch, 0:64], pst[:, 64:128]
                                    )
                        # Spread the data-ready early attention units between
                        # the Q/K/V sub-blocks so their exp-waits overlap
                        # prologue compute instead of serializing behind it.
                        early_advance(min(8 * qtr,
                                          exp_state["emitted"] + 3))
                    if qtr == 3:
                        early_advance(24)

            # ---- attention (flash-style, no max pass), with the output
            # projection interleaved per s-block so its matmuls and output
            # DMA hide under the ACT-bound attention stream ----------------
            with (
                tc.tile_pool(name="pssc", bufs=2, space="PSUM") as pscp,
                tc.tile_pool(name="pspr", bufs=2, space="PSUM") as pprp,
                tc.tile_pool(name="avs", bufs=2) as avpool,
            ):
                pending_proj = []
                last_av = [None]

                def emit_scores(u):
                    sb, pair, tch = u
                    kTp = kTp0 if pair == 0 else kTp1
                    s0, t0 = sb * SBLK, tch * 128
                    pssc = pscp.tile([128, SBLK], f32, name="pssc")
                    for nn in range(SBLK // 512):
                        nc.tensor.matmul(
                            pssc[:, nn * 512:(nn + 1) * 512],
                            kTp[:, t0:t0 + 128],
                            qT2[:, s0 + nn * 512:s0 + (nn + 1) * 512],
                            start=True, stop=True,
                        )
                    expT = exppool.tile([128, SBLK], f32r, name="expT")
                    nc.scalar.activation(expT[:], pssc[:], AF.Exp, scale=0.125)
                    return expT

                def emit_proj(sb, gate):
                    for scn in range(SBLK // 512):
                        s0 = sb * SBLK + scn * 512
                        for m in range(CC):
                            pspr = pprp.tile([128, 512], f32, name="pspr")
                            mm = nc.tensor.matmul(
                                pspr[:], wp_sb[:, m, :], outT2[:, s0:s0 + 512],
                                start=True, stop=True,
                            )
                            if gate is not None:
                                # Keep proj behind the attention stream so the
                                # norm chain (recip etc.) finishes off-PE first.
                                add_dep_helper(mm.ins, gate.ins, sync=False,
                                               reason="defer proj past boundary")
                            po = outpool.tile([128, 512], f32, name="po")
                            nc.vector.tensor_scalar_add(
                                po[:], pspr[:], pb_sb[:, m:m + 1]
                            )
                            nc.sync.dma_start(
                                out=o.ap()[m * 128:(m + 1) * 128, s0:s0 + 512],
                                in_=po[:],
                            )

                def emit_av(u, expT):
                    sb, pair, tch = u
                    p0 = pair * 64
                    v1 = v1_0 if pair == 0 else v1_1
                    if tch == 0:
                        psavs[(sb, pair)] = pavp.tile([65, SBLK], f32,
                                                      name="psav")
                    psav = psavs[(sb, pair)]
                    for nn in range(SBLK // 512):
                        last_av[0] = nc.tensor.matmul(
                            psav[:, nn * 512:(nn + 1) * 512],
                            v1[:, tch, :],
                            expT[:, nn * 512:(nn + 1) * 512],
                            start=(tch == 0), stop=(tch == TCH - 1),
                        )
                    if tch == TCH - 1:
                        # Move to SBUF immediately (frees the PSUM bank for
                        # the next s-block), then normalize from SBUF. Done
                        # in 512-wide halves so on the final s-block the
                        # reciprocal pipeline overlaps the projection.
                        avs = avpool.tile([65, SBLK], f32, name="avs")
                        nc.vector.tensor_copy(avs[:], psav[:])
                        nh = 4 if (sb == NSB - 1 and pair == 1) else 2
                        for hh in range(nh):
                            h0 = hh * (SBLK // nh)
                            h1 = h0 + SBLK // nh
                            recip = normpool.tile([1, SBLK // 2], f32,
                                                  name="recip")
                            nc.vector.reciprocal(recip[:, :h1 - h0],
                                                 avs[64:65, h0:h1])
                            rb = normpool.tile([64, SBLK // 2], f32, name="rb")
                            nc.gpsimd.partition_broadcast(rb[:, :h1 - h0],
                                                          recip[:, :h1 - h0])
                            nc.vector.tensor_mul(
                                outT2[p0:p0 + 64,
                                      sb * SBLK + h0:sb * SBLK + h1],
                                avs[0:64, h0:h1], rb[:, :h1 - h0]
                            )
                        if pair == 1:
                            # Delay the projection a few units so the norm
                            # chain (copy/recip/broadcast/mul on DVE+GPSIMD)
                            # finishes before the in-order PE reaches the
                            # proj matmuls.
                            pending_proj.append(sb)

                start_i = exp_state["emitted"]
                prev = exp_state["prev"]
                if start_i == 0:
                    prev = emit_scores(units[0])
                    start_i = 1
                for i in range(start_i, len(units)):
                    cur = emit_scores(units[i])
                    emit_av(units[i - 1], prev)
                    prev = cur
                    if pending_proj and (i % TCH) == 16:
                        emit_proj(pending_proj.pop(0), last_av[0])
                emit_av(units[-1], prev)
                for sb in pending_proj:
                    emit_proj(sb, None)
            ctx_psav.__exit__(None, None, None)

    nc.compile()
    return nc


def _prep_core_inputs(c, x, Wq, bq, Wk, bk, Wv, bv, Wp, bp):
    b = c // 4
    hs = 128 * (c % 4)

    def wslice_T(W):
        # W[hs:hs+128, :].T rearranged to [p, cc, d]
        return np.ascontiguousarray(
            W[hs:hs + 128, :].T.reshape(CC, 128, 128).transpose(1, 0, 2)
        )

    wp_arr = np.ascontiguousarray(
        Wp[:, hs:hs + 128].reshape(CC, 128, 128).transpose(2, 0, 1)
    )
    bqk_arr = np.ascontiguousarray(
        np.stack([bq[hs:hs + 128], bk[hs:hs + 128]], axis=1)
    ).astype(np.float32)
    vec = (bv[hs:hs + 128].astype(np.float64)
           @ Wp[:, hs:hs + 128].T.astype(np.float64)) + bp.astype(np.float64) / 4.0
    pbias_arr = np.ascontiguousarray(vec.reshape(CC, 128).T).astype(np.float32)

    return {
        "xb": np.ascontiguousarray(x[b].reshape(C, S)),
        "wq": wslice_T(Wq),
        "wk": wslice_T(Wk),
        "wv": wslice_T(Wv),
        "wp": wp_arr,
        "bqk": bqk_arr,
        "pbias": pbias_arr,
    }


def kernel(x, Wq, bq, Wk, bk, Wv, bv, Wp, bp):
    global LAST_EXEC_NS, LAST_RESULTS
    from concourse.bass_utils import run_bass_kernel_spmd

    x, Wq, bq, Wk, bk, Wv, bv, Wp, bp = (
        np.asarray(a, dtype=np.float32)
        for a in (x, Wq, bq, Wk, bk, Wv, bv, Wp, bp)
    )

    if "nc" not in _cached:
        _cached["nc"] = _build()
    nc = _cached["nc"]

    in_maps = [
        _prep_core_inputs(c, x, Wq, bq, Wk, bk, Wv, bv, Wp, bp)
        for c in range(NCORES)
    ]
    trace = bool(os.environ.get("BASS_TRACE"))
    res = run_bass_kernel_spmd(nc, in_maps, core_ids=list(range(NCORES)),
                               trace=trace)
    LAST_RESULTS = res
    LAST_EXEC_NS = res.exec_time_ns

    out = np.zeros((B, C, S), dtype=np.float32)
    for c in range(NCORES):
        out[c // 4] += res.results[c]["o"]
    return out.reshape(B, C, HH, WW)


# revision 29
# speedup vs baseline: 1.0099x; 1.0099x over previous
"""Multi-head self-attention (B=2, C=512, H=W=64, 8 heads) on 8 Trainium2 cores.

Sharding: data-parallel over B x head-parallel (2 heads/core). Core c handles
batch b = c//4 and heads {2*(c%4), 2*(c%4)+1} -- a contiguous 128-wide slice of
the 512-dim channel space.

Everything is laid out to avoid transposes entirely:
  - x[b] viewed as [C, S] is tok^T already (S = H*W = 4096 tokens).
  - q^T, k^T computed as [d2=128, S] (both heads stacked on partitions).
  - scores are computed TRANSPOSED: scoresT[t, s] = sum_d kT[d,t] qT[d,s],
    so softmax's exp is along the free dim and attn.V contracts t on partitions.
  - No max-subtraction needed: scores/8 ~ N(0, 0.33), exp never overflows.
  - The softmax denominator is obtained by appending a ones-column to V:
    one matmul yields both attn.V and sum(exp) rows.
  - Normalization (1/denom, varies along free dim) commutes with nothing on
    the partition axis, so it is done with a GPSIMD partition-broadcast plus
    a DVE multiply.
  - Output projection is input-column sharded: each core contributes
    attn_out[:, d_slice] @ Wp[:, d_slice].T; host sums the 4 partials per b.
    The V bias contribution (bv_slice @ WpT_slice) is folded into a
    host-precomputed per-core projection bias, so V needs no on-device bias.

All matmuls run as float32r (single-pass reduced-precision fp32, ~1.5e-4 max
rel err, ~3x faster than 2-pass fp32). exp runs on the scalar (ACT) engine
(33.5M exps/core ~ 276us busy); the PE stream (scores + attn.V, ~1.15us per
128-key x 1024-query unit) is software-pipelined against it: scores(t+1) is
issued before attn.V(t) so the PE never stalls on exp and the HAM clock
throttle stays disengaged (cold-clock matmuls are ~2x slower).

Measured on 8 axon-attached trn2 cores: ~390us HW exec, overall rel err
~6.8e-5 vs the fp32 jax reference (error entirely from f32r rounding).
"""

import os
import sys

sys.path.insert(0, "/opt/trn_rl_repo")

import numpy as np

NCORES = 8
B, C, HH, WW = 2, 512, 64, 64
S = HH * WW            # 4096 tokens
NH, D = 8, 64          # heads, head dim
DSL = 128              # per-core d-slice (2 heads)
CC = C // 128          # 4 contraction chunks
TCH = S // 128         # 32 key chunks
SBLK = 1024            # queries per attention block
NSB = S // SBLK        # 4
NSC = S // 512         # 8 (512-wide matmul slices)

_cached = {}

LAST_EXEC_NS = None
LAST_RESULTS = None


def _build():
    import concourse.mybir as mybir
    import concourse.tile as tile
    from bass_rust import add_dep_helper
    from concourse import bacc

    f32 = mybir.dt.float32
    f32r = mybir.dt.float32r
    AF = mybir.ActivationFunctionType

    nc = bacc.Bacc("TRN2", target_bir_lowering=False, debug=False,
                   num_devices=NCORES)

    xb = nc.dram_tensor("xb", [C, S], f32r, kind="ExternalInput")
    wq = nc.dram_tensor("wq", [128, CC, 128], f32r, kind="ExternalInput")
    wk = nc.dram_tensor("wk", [128, CC, 128], f32r, kind="ExternalInput")
    wv = nc.dram_tensor("wv", [128, CC, 128], f32r, kind="ExternalInput")
    wp = nc.dram_tensor("wp", [128, CC, 128], f32r, kind="ExternalInput")
    bqk = nc.dram_tensor("bqk", [128, 2], f32, kind="ExternalInput")
    pbias = nc.dram_tensor("pbias", [128, CC], f32, kind="ExternalInput")
    o = nc.dram_tensor("o", [C, S], f32, kind="ExternalOutput")

    with tile.TileContext(nc) as tc:
        with (
            tc.tile_pool(name="weights", bufs=1) as wpool,
            tc.tile_pool(name="tok", bufs=1) as tokpool,
            tc.tile_pool(name="qkv", bufs=1) as qkvpool,
            tc.tile_pool(name="exps", bufs=4) as exppool,
            tc.tile_pool(name="norm", bufs=4) as normpool,
            tc.tile_pool(name="outp", bufs=3) as outpool,
        ):
            wq_sb = wpool.tile([128, CC, 128], f32r, name="wq_sb")
            nc.sync.dma_start(out=wq_sb[:], in_=wq.ap())
            wk_sb = wpool.tile([128, CC, 128], f32r, name="wk_sb")
            nc.sync.dma_start(out=wk_sb[:], in_=wk.ap())
            wv_sb = wpool.tile([128, CC, 128], f32r, name="wv_sb")
            nc.sync.dma_start(out=wv_sb[:], in_=wv.ap())
            wp_sb = wpool.tile([128, CC, 128], f32r, name="wp_sb")
            nc.sync.dma_start(out=wp_sb[:], in_=wp.ap())
            bqk_sb = wpool.tile([128, 2], f32, name="bqk_sb")
            nc.sync.dma_start(out=bqk_sb[:], in_=bqk.ap())
            pb_sb = wpool.tile([128, CC], f32, name="pb_sb")
            nc.sync.dma_start(out=pb_sb[:], in_=pbias.ap())

            # tok^T in [partition, c_chunk, s] layout; DMA rearranges rows.
            tok_sb = tokpool.tile([128, CC, S], f32r, name="tok_sb")
            x_re = xb.ap().rearrange("(cc p) s -> p cc s", p=128)
            for qtr in range(4):
                for cc in range(CC):
                    for hf in range(2):
                        a = qtr * (S // 4) + hf * (S // 8)
                        sl = slice(a, a + S // 8)
                        nc.sync.dma_start(out=tok_sb[:, cc, sl],
                                          in_=x_re[:, cc, sl])

            qT2 = qkvpool.tile([128, S], f32r, name="qT2")
            # k^T is stored twice, zero-padded to a full K=128 contraction:
            # kTp0 = [kT_pair0; 0], kTp1 = [0; kT_pair1]. A K=128 f32r matmul
            # streams 2x faster than K=64 (measured 336 vs 526 ns), and the
            # zero rows nullify the other pair's rows of the shared qT2.
            kTp0 = qkvpool.tile([128, S], f32r, name="kTp0")
            kTp1 = qkvpool.tile([128, S], f32r, name="kTp1")
            zer32 = qkvpool.tile([64, 512], f32, name="zer32")
            nc.vector.memset(zer32[:], 0.0)
            nc.vector.tensor_copy(kTp0[64:128, 0:512], zer32[:])
            for j in range(1, 8):
                nc.vector.tensor_copy(kTp0[64:128, j * 512:(j + 1) * 512], zer32[:])
            for j in range(8):
                nc.vector.tensor_copy(kTp1[0:64, j * 512:(j + 1) * 512], zer32[:])
            # v with a ones column per key-chunk, per pair: [t, chunk, 65]
            v1_0 = qkvpool.tile([128, TCH, 65], f32r, name="v1_0")
            v1_1 = qkvpool.tile([128, TCH, 65], f32r, name="v1_1")
            ones32 = qkvpool.tile([128, TCH], f32, name="ones32")
            nc.vector.memset(ones32[:], 1.0)
            nc.vector.tensor_copy(v1_0[:, :, 64], ones32[:])
            nc.vector.tensor_copy(v1_1[:, :, 64], ones32[:])

            outT2 = qkvpool.tile([128, S], f32r, name="outT2")

            # ---- fused Q/K/V prologue, quarter-major so compute chases
            # the x DMA. V is computed transposed (efficient N=512 matmuls)
            # and flipped into [t, d] layout with PE transposes.
            ident = qkvpool.tile([128, 128], f32, name="ident")
            from concourse.masks import make_identity
            make_identity(nc, ident[:])
            ctx_psav = tc.tile_pool(name="psav", bufs=1, space="PSUM")
            pavp = ctx_psav.__enter__()
            psavs = {}
            exp_state = {"emitted": 0, "prev": None, "early": []}
            with (
                tc.tile_pool(name="psqk", bufs=2, space="PSUM") as pqkp,
                tc.tile_pool(name="pst", bufs=2, space="PSUM") as pstp,
                tc.tile_pool(name="pssce", bufs=1, space="PSUM") as pscep,
                tc.tile_pool(name="vt", bufs=2) as vtpool,
            ):
                units = [(sb, pair, tch)
                         for sb in range(NSB) for pair in range(2)
                         for tch in range(TCH)]

                def early_scores(u):
                    sb, pair, tch = u
                    kTp = kTp0 if pair == 0 else kTp1
                    s0, t0 = sb * SBLK, tch * 128
                    pssc = pscep.tile([128, SBLK], f32, name="pssce")
                    for nn in range(SBLK // 512):
                        nc.tensor.matmul(
                            pssc[:, nn * 512:(nn + 1) * 512],
                            kTp[:, t0:t0 + 128],
                            qT2[:, s0 + nn * 512:s0 + (nn + 1) * 512],
                            start=True, stop=True,
                        )
                    expT = exppool.tile([128, SBLK], f32r, name="expT")
                    nc.scalar.activation(expT[:], pssc[:], AF.Exp, scale=0.125)
                    return expT

                def early_av(u, expT):
                    sb, pair, tch = u
                    v1 = v1_0 if pair == 0 else v1_1
                    if tch == 0:
                        psavs[(sb, pair)] = pavp.tile([65, SBLK], f32,
                                                      name="psav")
                    psav = psavs[(sb, pair)]
                    for nn in range(SBLK // 512):
                        nc.tensor.matmul(
                            psav[:, nn * 512:(nn + 1) * 512],
                            v1[:, tch, :],
                            expT[:, nn * 512:(nn + 1) * 512],
                            start=(tch == 0), stop=(tch == TCH - 1),
                        )

                def early_advance(k):
                    st = exp_state
                    while st["emitted"] < k:
                        i = st["emitted"]
                        cur = early_scores(units[i])
                        if i > 0:
                            early_av(units[i - 1], st["prev"])
                        st["prev"] = cur
                        st["emitted"] = i + 1

                for qtr in range(4):
                    for which in range(3):
                        w_sb = (wq_sb, wk_sb, wv_sb)[which]
                        for scq in range(2):
                            sc = qtr * 2 + scq
                            s0 = sc * 512
                            psqk = pqkp.tile([128, 512], f32, name="psqk")
                            for cc in range(CC):
                                nc.tensor.matmul(
                                    psqk[:],
                                    w_sb[:, cc, :],
                                    tok_sb[:, cc, s0:s0 + 512],
                                    start=(cc == 0), stop=(cc == CC - 1),
                                )
                            if which == 0:
                                nc.vector.tensor_scalar_add(
                                    qT2[:, s0:s0 + 512], psqk[:], bqk_sb[:, 0:1]
                                )
                            elif which == 1:
                                nc.vector.tensor_scalar_add(
                                    kTp0[0:64, s0:s0 + 512], psqk[0:64, :],
                                    bqk_sb[0:64, 1:2]
                                )
                                nc.vector.tensor_scalar_add(
                                    kTp1[64:128, s0:s0 + 512], psqk[64:128, :],
                                    bqk_sb[64:128, 1:2]
                                )
                            else:
                                vt = vtpool.tile([128, 512], f32r, name="vt")
                                nc.vector.tensor_copy(vt[:], psqk[:])
                                for tt in range(4):
                                    tch = sc * 4 + tt
                                    pst = pstp.tile([128, 128], f32, name="pst")
                                    nc.tensor.transpose(
                                        pst[:],
                                        vt[:, tt * 128:(tt + 1) * 128].bitcast(f32),
                                        ident[:],
                                    )
                                    nc.vector.tensor_copy(
                                        v1_0[:, tch, 0:64], pst[:, 0:64]
                                    )
                                    nc.vector.tensor_copy(
                                        v1_1[:, tch, 0:64], pst[:, 64:128]
                                    )
                    if qtr < 3:
                        early_advance(8 * (qtr + 1))

            # ---- attention (flash-style, no max pass), with the output
            # projection interleaved per s-block so its matmuls and output
            # DMA hide under the ACT-bound attention stream ----------------
            with (
                tc.tile_pool(name="pssc", bufs=2, space="PSUM") as pscp,
                tc.tile_pool(name="pspr", bufs=2, space="PSUM") as pprp,
                tc.tile_pool(name="avs", bufs=2) as avpool,
            ):
                pending_proj = []
                last_av = [None]

                def emit_scores(u):
                    sb, pair, tch = u
                    kTp = kTp0 if pair == 0 else kTp1
                    s0, t0 = sb * SBLK, tch * 128
                    pssc = pscp.tile([128, SBLK], f32, name="pssc")
                    for nn in range(SBLK // 512):
                        nc.tensor.matmul(
                            pssc[:, nn * 512:(nn + 1) * 512],
                            kTp[:, t0:t0 + 128],
                            qT2[:, s0 + nn * 512:s0 + (nn + 1) * 512],
                            start=True, stop=True,
                        )
                    expT = exppool.tile([128, SBLK], f32r, name="expT")
                    nc.scalar.activation(expT[:], pssc[:], AF.Exp, scale=0.125)
                    return expT

                def emit_proj(sb, gate):
                    for scn in range(SBLK // 512):
                        s0 = sb * SBLK + scn * 512
                        for m in range(CC):
                            pspr = pprp.tile([128, 512], f32, name="pspr")
                            mm = nc.tensor.matmul(
                                pspr[:], wp_sb[:, m, :], outT2[:, s0:s0 + 512],
                                start=True, stop=True,
                            )
                            if gate is not None:
                                # Keep proj behind the attention stream so the
                                # norm chain (recip etc.) finishes off-PE first.
                                add_dep_helper(mm.ins, gate.ins, sync=False,
                                               reason="defer proj past boundary")
                            po = outpool.tile([128, 512], f32, name="po")
                            nc.vector.tensor_scalar_add(
                                po[:], pspr[:], pb_sb[:, m:m + 1]
                            )
                            nc.sync.dma_start(
                                out=o.ap()[m * 128:(m + 1) * 128, s0:s0 + 512],
                                in_=po[:],
                            )

                def emit_av(u, expT):
                    sb, pair, tch = u
                    p0 = pair * 64
                    v1 = v1_0 if pair == 0 else v1_1
                    if tch == 0:
                        psavs[(sb, pair)] = pavp.tile([65, SBLK], f32,
                                                      name="psav")
                    psav = psavs[(sb, pair)]
                    for nn in range(SBLK // 512):
                        last_av[0] = nc.tensor.matmul(
                            psav[:, nn * 512:(nn + 1) * 512],
                            v1[:, tch, :],
                            expT[:, nn * 512:(nn + 1) * 512],
                            start=(tch == 0), stop=(tch == TCH - 1),
                        )
                    if tch == TCH - 1:
                        # Move to SBUF immediately (frees the PSUM bank for
                        # the next s-block), then normalize from SBUF. Done
                        # in 512-wide halves so on the final s-block the
                        # reciprocal pipeline overlaps the projection.
                        avs = avpool.tile([65, SBLK], f32, name="avs")
                        nc.vector.tensor_copy(avs[:], psav[:])
                        nh = 4 if (sb == NSB - 1 and pair == 1) else 2
                        for hh in range(nh):
                            h0 = hh * (SBLK // nh)
                            h1 = h0 + SBLK // nh
                            recip = normpool.tile([1, SBLK // 2], f32,
                                                  name="recip")
                            nc.vector.reciprocal(recip[:, :h1 - h0],
                                                 avs[64:65, h0:h1])
                            rb = normpool.tile([64, SBLK // 2], f32, name="rb")
                            nc.gpsimd.partition_broadcast(rb[:, :h1 - h0],
                                                          recip[:, :h1 - h0])
                            nc.vector.tensor_mul(
                                outT2[p0:p0 + 64,
                                      sb * SBLK + h0:sb * SBLK + h1],
                                avs[0:64, h0:h1], rb[:, :h1 - h0]
                            )
                        if pair == 1:
                            # Delay the projection a few units so the norm
                            # chain (copy/recip/broadcast/mul on DVE+GPSIMD)
                            # finishes before the in-order PE reaches the
                            # proj matmuls.
                            pending_proj.append(sb)

                start_i = exp_state["emitted"]
                prev = exp_state["prev"]
                if start_i == 0:
                    prev = emit_scores(units[0])
                    start_i = 1
                for i in range(start_i, len(units)):
                    cur = emit_scores(units[i])
                    emit_av(units[i - 1], prev)
                    prev = cur
                    if pending_proj and (i % TCH) == 16:
                        emit_proj(pending_proj.pop(0), last_av[0])
                emit_av(units[-1], prev)
                for sb in pending_proj:
                    emit_proj(sb, None)
            ctx_psav.__exit__(None, None, None)

    nc.compile()
    return nc


def _prep_core_inputs(c, x, Wq, bq, Wk, bk, Wv, bv, Wp, bp):
    b = c // 4
    hs = 128 * (c % 4)

    def wslice_T(W):
        # W[hs:hs+128, :].T rearranged to [p, cc, d]
        return np.ascontiguousarray(
            W[hs:hs + 128, :].T.reshape(CC, 128, 128).transpose(1, 0, 2)
        )

    wp_arr = np.ascontiguousarray(
        Wp[:, hs:hs + 128].reshape(CC, 128, 128).transpose(2, 0, 1)
    )
    bqk_arr = np.ascontiguousarray(
        np.stack([bq[hs:hs + 128], bk[hs:hs + 128]], axis=1)
    ).astype(np.float32)
    vec = (bv[hs:hs + 128].astype(np.float64)
           @ Wp[:, hs:hs + 128].T.astype(np.float64)) + bp.astype(np.float64) / 4.0
    pbias_arr = np.ascontiguousarray(vec.reshape(CC, 128).T).astype(np.float32)

    return {
        "xb": np.ascontiguousarray(x[b].reshape(C, S)),
        "wq": wslice_T(Wq),
        "wk": wslice_T(Wk),
        "wv": wslice_T(Wv),
        "wp": wp_arr,
        "bqk": bqk_arr,
        "pbias": pbias_arr,
    }


def kernel(x, Wq, bq, Wk, bk, Wv, bv, Wp, bp):
    global LAST_EXEC_NS, LAST_RESULTS
    from concourse.bass_utils import run_bass_kernel_spmd

    x, Wq, bq, Wk, bk, Wv, bv, Wp, bp = (
        np.asarray(a, dtype=np.float32)
        for a in (x, Wq, bq, Wk, bk, Wv, bv, Wp, bp)
    )

    if "nc" not in _cached:
        _cached["nc"] = _build()
    nc = _cached["nc"]

    in_maps = [
        _prep_core_inputs(c, x, Wq, bq, Wk, bk, Wv, bv, Wp, bp)
        for c in range(NCORES)
    ]
    trace = bool(os.environ.get("BASS_TRACE"))
    res = run_bass_kernel_spmd(nc, in_maps, core_ids=list(range(NCORES)),
                               trace=trace)
    LAST_RESULTS = res
    LAST_EXEC_NS = res.exec_time_ns

    out = np.zeros((B, C, S), dtype=np.float32)
    for c in range(NCORES):
        out[c // 4] += res.results[c]["o"]
    return out.reshape(B, C, HH, WW)


# revision 30
# speedup vs baseline: 1.0110x; 1.0010x over previous
"""Multi-head self-attention (B=2, C=512, H=W=64, 8 heads) on 8 Trainium2 cores.

Sharding: data-parallel over B x head-parallel (2 heads/core). Core c handles
batch b = c//4 and heads {2*(c%4), 2*(c%4)+1} -- a contiguous 128-wide slice of
the 512-dim channel space.

Everything is laid out to avoid transposes entirely:
  - x[b] viewed as [C, S] is tok^T already (S = H*W = 4096 tokens).
  - q^T, k^T computed as [d2=128, S] (both heads stacked on partitions).
  - scores are computed TRANSPOSED: scoresT[t, s] = sum_d kT[d,t] qT[d,s],
    so softmax's exp is along the free dim and attn.V contracts t on partitions.
  - No max-subtraction needed: scores/8 ~ N(0, 0.33), exp never overflows.
  - The softmax denominator is obtained by appending a ones-column to V:
    one matmul yields both attn.V and sum(exp) rows.
  - Normalization (1/denom, varies along free dim) commutes with nothing on
    the partition axis, so it is done with a GPSIMD partition-broadcast plus
    a DVE multiply.
  - Output projection is input-column sharded: each core contributes
    attn_out[:, d_slice] @ Wp[:, d_slice].T; host sums the 4 partials per b.
    The V bias contribution (bv_slice @ WpT_slice) is folded into a
    host-precomputed per-core projection bias, so V needs no on-device bias.

All matmuls run as float32r (single-pass reduced-precision fp32, ~1.5e-4 max
rel err, ~3x faster than 2-pass fp32). exp runs on the scalar (ACT) engine
(33.5M exps/core ~ 276us busy); the PE stream (scores + attn.V, ~1.15us per
128-key x 1024-query unit) is software-pipelined against it: scores(t+1) is
issued before attn.V(t) so the PE never stalls on exp and the HAM clock
throttle stays disengaged (cold-clock matmuls are ~2x slower).

Measured on 8 axon-attached trn2 cores: ~381us HW exec, overall rel err
~6.8e-5 vs the fp32 jax reference (error entirely from f32r rounding).
"""

import os
import sys

sys.path.insert(0, "/opt/trn_rl_repo")

import numpy as np

NCORES = 8
B, C, HH, WW = 2, 512, 64, 64
S = HH * WW            # 4096 tokens
NH, D = 8, 64          # heads, head dim
DSL = 128              # per-core d-slice (2 heads)
CC = C // 128          # 4 contraction chunks
TCH = S // 128         # 32 key chunks
SBLK = 1024            # queries per attention block
NSB = S // SBLK        # 4
NSC = S // 512         # 8 (512-wide matmul slices)

_cached = {}

LAST_EXEC_NS = None
LAST_RESULTS = None


def _build():
    import concourse.mybir as mybir
    import concourse.tile as tile
    from bass_rust import add_dep_helper
    from concourse import bacc

    f32 = mybir.dt.float32
    f32r = mybir.dt.float32r
    AF = mybir.ActivationFunctionType

    nc = bacc.Bacc("TRN2", target_bir_lowering=False, debug=False,
                   num_devices=NCORES)

    xb = nc.dram_tensor("xb", [C, S], f32r, kind="ExternalInput")
    wq = nc.dram_tensor("wq", [128, CC, 128], f32r, kind="ExternalInput")
    wk = nc.dram_tensor("wk", [128, CC, 128], f32r, kind="ExternalInput")
    wv = nc.dram_tensor("wv", [128, CC, 128], f32r, kind="ExternalInput")
    wp = nc.dram_tensor("wp", [128, CC, 128], f32r, kind="ExternalInput")
    bqk = nc.dram_tensor("bqk", [128, 2], f32, kind="ExternalInput")
    pbias = nc.dram_tensor("pbias", [128, CC], f32, kind="ExternalInput")
    o = nc.dram_tensor("o", [C, S], f32, kind="ExternalOutput")

    with tile.TileContext(nc) as tc:
        with (
            tc.tile_pool(name="weights", bufs=1) as wpool,
            tc.tile_pool(name="tok", bufs=1) as tokpool,
            tc.tile_pool(name="qkv", bufs=1) as qkvpool,
            tc.tile_pool(name="exps", bufs=4) as exppool,
            tc.tile_pool(name="norm", bufs=4) as normpool,
            tc.tile_pool(name="outp", bufs=3) as outpool,
        ):
            wq_sb = wpool.tile([128, CC, 128], f32r, name="wq_sb")
            nc.sync.dma_start(out=wq_sb[:], in_=wq.ap())
            wk_sb = wpool.tile([128, CC, 128], f32r, name="wk_sb")
            nc.sync.dma_start(out=wk_sb[:], in_=wk.ap())
            wv_sb = wpool.tile([128, CC, 128], f32r, name="wv_sb")
            nc.sync.dma_start(out=wv_sb[:], in_=wv.ap())
            wp_sb = wpool.tile([128, CC, 128], f32r, name="wp_sb")
            nc.sync.dma_start(out=wp_sb[:], in_=wp.ap())
            bqk_sb = wpool.tile([128, 2], f32, name="bqk_sb")
            nc.sync.dma_start(out=bqk_sb[:], in_=bqk.ap())
            pb_sb = wpool.tile([128, CC], f32, name="pb_sb")
            nc.sync.dma_start(out=pb_sb[:], in_=pbias.ap())

            # tok^T in [partition, c_chunk, s] layout; DMA rearranges rows.
            tok_sb = tokpool.tile([128, CC, S], f32r, name="tok_sb")
            x_re = xb.ap().rearrange("(cc p) s -> p cc s", p=128)
            for qtr in range(4):
                for cc in range(CC):
                    for hf in range(2):
                        a = qtr * (S // 4) + hf * (S // 8)
                        sl = slice(a, a + S // 8)
                        nc.sync.dma_start(out=tok_sb[:, cc, sl],
                                          in_=x_re[:, cc, sl])

            qT2 = qkvpool.tile([128, S], f32r, name="qT2")
            # k^T is stored twice, zero-padded to a full K=128 contraction:
            # kTp0 = [kT_pair0; 0], kTp1 = [0; kT_pair1]. A K=128 f32r matmul
            # streams 2x faster than K=64 (measured 336 vs 526 ns), and the
            # zero rows nullify the other pair's rows of the shared qT2.
            kTp0 = qkvpool.tile([128, S], f32r, name="kTp0")
            kTp1 = qkvpool.tile([128, S], f32r, name="kTp1")
            zer32 = qkvpool.tile([64, 512], f32, name="zer32")
            nc.vector.memset(zer32[:], 0.0)
            nc.vector.tensor_copy(kTp0[64:128, 0:512], zer32[:])
            for j in range(1, 8):
                nc.vector.tensor_copy(kTp0[64:128, j * 512:(j + 1) * 512], zer32[:])
            for j in range(8):
                nc.vector.tensor_copy(kTp1[0:64, j * 512:(j + 1) * 512], zer32[:])
            # v with a ones column per key-chunk, per pair: [t, chunk, 65]
            v1_0 = qkvpool.tile([128, TCH, 65], f32r, name="v1_0")
            v1_1 = qkvpool.tile([128, TCH, 65], f32r, name="v1_1")
            ones32 = qkvpool.tile([128, TCH], f32, name="ones32")
            nc.vector.memset(ones32[:], 1.0)
            nc.vector.tensor_copy(v1_0[:, :, 64], ones32[:])
            nc.vector.tensor_copy(v1_1[:, :, 64], ones32[:])

            outT2 = qkvpool.tile([128, S], f32r, name="outT2")

            # ---- fused Q/K/V prologue, quarter-major so compute chases
            # the x DMA. V is computed transposed (efficient N=512 matmuls)
            # and flipped into [t, d] layout with PE transposes.
            ident = qkvpool.tile([128, 128], f32, name="ident")
            from concourse.masks import make_identity
            make_identity(nc, ident[:])
            ctx_psav = tc.tile_pool(name="psav", bufs=1, space="PSUM")
            pavp = ctx_psav.__enter__()
            psavs = {}
            exp_state = {"emitted": 0, "prev": None, "early": []}
            with (
                tc.tile_pool(name="psqk", bufs=2, space="PSUM") as pqkp,
                tc.tile_pool(name="pst", bufs=2, space="PSUM") as pstp,
                tc.tile_pool(name="pssce", bufs=1, space="PSUM") as pscep,
                tc.tile_pool(name="vt", bufs=2) as vtpool,
            ):
                units = [(sb, pair, tch)
                         for sb in range(NSB) for pair in range(2)
                         for tch in range(TCH)]

                def early_scores(u):
                    sb, pair, tch = u
                    kTp = kTp0 if pair == 0 else kTp1
                    s0, t0 = sb * SBLK, tch * 128
                    pssc = pscep.tile([128, SBLK], f32, name="pssce")
                    for nn in range(SBLK // 512):
                        nc.tensor.matmul(
                            pssc[:, nn * 512:(nn + 1) * 512],
                            kTp[:, t0:t0 + 128],
                            qT2[:, s0 + nn * 512:s0 + (nn + 1) * 512],
                            start=True, stop=True,
                        )
                    expT = exppool.tile([128, SBLK], f32r, name="expT")
                    nc.scalar.activation(expT[:], pssc[:], AF.Exp, scale=0.125)
                    return expT

                def early_av(u, expT):
                    sb, pair, tch = u
                    v1 = v1_0 if pair == 0 else v1_1
                    if tch == 0:
                        psavs[(sb, pair)] = pavp.tile([65, SBLK], f32,
                                                      name="psav")
                    psav = psavs[(sb, pair)]
                    for nn in range(SBLK // 512):
                        nc.tensor.matmul(
                            psav[:, nn * 512:(nn + 1) * 512],
                            v1[:, tch, :],
                            expT[:, nn * 512:(nn + 1) * 512],
                            start=(tch == 0), stop=(tch == TCH - 1),
                        )

                def early_advance(k):
                    st = exp_state
                    while st["emitted"] < k:
                        i = st["emitted"]
                        cur = early_scores(units[i])
                        if i > 0:
                            early_av(units[i - 1], st["prev"])
                        st["prev"] = cur
                        st["emitted"] = i + 1

                for qtr in range(4):
                    for which in range(3):
                        w_sb = (wq_sb, wk_sb, wv_sb)[which]
                        for scq in range(2):
                            sc = qtr * 2 + scq
                            s0 = sc * 512
                            psqk = pqkp.tile([128, 512], f32, name="psqk")
                            for cc in range(CC):
                                nc.tensor.matmul(
                                    psqk[:],
                                    w_sb[:, cc, :],
                                    tok_sb[:, cc, s0:s0 + 512],
                                    start=(cc == 0), stop=(cc == CC - 1),
                                )
                            if which == 0:
                                nc.vector.tensor_scalar_add(
                                    qT2[:, s0:s0 + 512], psqk[:], bqk_sb[:, 0:1]
                                )
                            elif which == 1:
                                nc.vector.tensor_scalar_add(
                                    kTp0[0:64, s0:s0 + 512], psqk[0:64, :],
                                    bqk_sb[0:64, 1:2]
                                )
                                nc.vector.tensor_scalar_add(
                                    kTp1[64:128, s0:s0 + 512], psqk[64:128, :],
                                    bqk_sb[64:128, 1:2]
                                )
                            else:
                                vt = vtpool.tile([128, 512], f32r, name="vt")
                                nc.vector.tensor_copy(vt[:], psqk[:])
                                for tt in range(4):
                                    tch = sc * 4 + tt
                                    pst = pstp.tile([128, 128], f32, name="pst")
                                    nc.tensor.transpose(
                                        pst[:],
                                        vt[:, tt * 128:(tt + 1) * 128].bitcast(f32),
                                        ident[:],
                                    )
                                    nc.vector.tensor_copy(
                                        v1_0[:, tch, 0:64], pst[:, 0:64]
                                    )
                                    nc.vector.tensor_copy(
                                        v1_1[:, tch, 0:64], pst[:, 64:128]
                                    )
                    if qtr < 3:
                        early_advance(8 * (qtr + 1))

            # ---- attention (flash-style, no max pass), with the output
            # projection interleaved per s-block so its matmuls and output
            # DMA hide under the ACT-bound attention stream ----------------
            with (
                tc.tile_pool(name="pssc", bufs=2, space="PSUM") as pscp,
                tc.tile_pool(name="pspr", bufs=2, space="PSUM") as pprp,
                tc.tile_pool(name="avs", bufs=2) as avpool,
            ):
                pending_proj = []
                last_av = [None]

                def emit_scores(u):
                    sb, pair, tch = u
                    kTp = kTp0 if pair == 0 else kTp1
                    s0, t0 = sb * SBLK, tch * 128
                    pssc = pscp.tile([128, SBLK], f32, name="pssc")
                    for nn in range(SBLK // 512):
                        nc.tensor.matmul(
                            pssc[:, nn * 512:(nn + 1) * 512],
                            kTp[:, t0:t0 + 128],
                            qT2[:, s0 + nn * 512:s0 + (nn + 1) * 512],
                            start=True, stop=True,
                        )
                    expT = exppool.tile([128, SBLK], f32r, name="expT")
                    nc.scalar.activation(expT[:], pssc[:], AF.Exp, scale=0.125)
                    return expT

                def emit_proj(sb, gate):
                    for scn in range(SBLK // 512):
                        s0 = sb * SBLK + scn * 512
                        for m in range(CC):
                            pspr = pprp.tile([128, 512], f32, name="pspr")
                            mm = nc.tensor.matmul(
                                pspr[:], wp_sb[:, m, :], outT2[:, s0:s0 + 512],
                                start=True, stop=True,
                            )
                            if gate is not None:
                                # Keep proj behind the attention stream so the
                                # norm chain (recip etc.) finishes off-PE first.
                                add_dep_helper(mm.ins, gate.ins, sync=False,
                                               reason="defer proj past boundary")
                            po = outpool.tile([128, 512], f32, name="po")
                            nc.vector.tensor_scalar_add(
                                po[:], pspr[:], pb_sb[:, m:m + 1]
                            )
                            nc.sync.dma_start(
                                out=o.ap()[m * 128:(m + 1) * 128, s0:s0 + 512],
                                in_=po[:],
                            )

                def emit_av(u, expT):
                    sb, pair, tch = u
                    p0 = pair * 64
                    v1 = v1_0 if pair == 0 else v1_1
                    if tch == 0:
                        psavs[(sb, pair)] = pavp.tile([65, SBLK], f32,
                                                      name="psav")
                    psav = psavs[(sb, pair)]
                    for nn in range(SBLK // 512):
                        last_av[0] = nc.tensor.matmul(
                            psav[:, nn * 512:(nn + 1) * 512],
                            v1[:, tch, :],
                            expT[:, nn * 512:(nn + 1) * 512],
                            start=(tch == 0), stop=(tch == TCH - 1),
                        )
                    if tch == TCH - 1:
                        # Move to SBUF immediately (frees the PSUM bank for
                        # the next s-block), then normalize from SBUF. Done
                        # in 512-wide halves so on the final s-block the
                        # reciprocal pipeline overlaps the projection.
                        avs = avpool.tile([65, SBLK], f32, name="avs")
                        nc.vector.tensor_copy(avs[:], psav[:])
                        nh = 4 if (sb == NSB - 1 and pair == 1) else 2
                        for hh in range(nh):
                            h0 = hh * (SBLK // nh)
                            h1 = h0 + SBLK // nh
                            recip = normpool.tile([1, SBLK // 2], f32,
                                                  name="recip")
                            nc.vector.reciprocal(recip[:, :h1 - h0],
                                                 avs[64:65, h0:h1])
                            rb = normpool.tile([64, SBLK // 2], f32, name="rb")
                            nc.gpsimd.partition_broadcast(rb[:, :h1 - h0],
                                                          recip[:, :h1 - h0])
                            nc.vector.tensor_mul(
                                outT2[p0:p0 + 64,
                                      sb * SBLK + h0:sb * SBLK + h1],
                                avs[0:64, h0:h1], rb[:, :h1 - h0]
                            )
                        if pair == 1:
                            # Delay the projection a few units so the norm
                            # chain (copy/recip/broadcast/mul on DVE+GPSIMD)
                            # finishes before the in-order PE reaches the
                            # proj matmuls.
                            pending_proj.append(sb)

                start_i = exp_state["emitted"]
                prev = exp_state["prev"]
                if start_i == 0:
                    prev = emit_scores(units[0])
                    start_i = 1
                for i in range(start_i, len(units)):
                    cur = emit_scores(units[i])
                    emit_av(units[i - 1], prev)
                    prev = cur
                    if pending_proj and (i % TCH) == 16:
                        emit_proj(pending_proj.pop(0), last_av[0])
                emit_av(units[-1], prev)
                for sb in pending_proj:
                    emit_proj(sb, None)
            ctx_psav.__exit__(None, None, None)

    nc.compile()
    return nc


def _prep_core_inputs(c, x, Wq, bq, Wk, bk, Wv, bv, Wp, bp):
    b = c // 4
    hs = 128 * (c % 4)

    def wslice_T(W):
        # W[hs:hs+128, :].T rearranged to [p, cc, d]
        return np.ascontiguousarray(
            W[hs:hs + 128, :].T.reshape(CC, 128, 128).transpose(1, 0, 2)
        )

    wp_arr = np.ascontiguousarray(
        Wp[:, hs:hs + 128].reshape(CC, 128, 128).transpose(2, 0, 1)
    )
    bqk_arr = np.ascontiguousarray(
        np.stack([bq[hs:hs + 128], bk[hs:hs + 128]], axis=1)
    ).astype(np.float32)
    vec = (bv[hs:hs + 128].astype(np.float64)
           @ Wp[:, hs:hs + 128].T.astype(np.float64)) + bp.astype(np.float64) / 4.0
    pbias_arr = np.ascontiguousarray(vec.reshape(CC, 128).T).astype(np.float32)

    return {
        "xb": np.ascontiguousarray(x[b].reshape(C, S)),
        "wq": wslice_T(Wq),
        "wk": wslice_T(Wk),
        "wv": wslice_T(Wv),
        "wp": wp_arr,
        "bqk": bqk_arr,
        "pbias": pbias_arr,
    }


def kernel(x, Wq, bq, Wk, bk, Wv, bv, Wp, bp):
    global LAST_EXEC_NS, LAST_RESULTS
    from concourse.bass_utils import run_bass_kernel_spmd

    x, Wq, bq, Wk, bk, Wv, bv, Wp, bp = (
        np.asarray(a, dtype=np.float32)
        for a in (x, Wq, bq, Wk, bk, Wv, bv, Wp, bp)
    )

    if "nc" not in _cached:
        _cached["nc"] = _build()
    nc = _cached["nc"]

    in_maps = [
        _prep_core_inputs(c, x, Wq, bq, Wk, bk, Wv, bv, Wp, bp)
        for c in range(NCORES)
    ]
    trace = bool(os.environ.get("BASS_TRACE"))
    res = run_bass_kernel_spmd(nc, in_maps, core_ids=list(range(NCORES)),
                               trace=trace)
    LAST_RESULTS = res
    LAST_EXEC_NS = res.exec_time_ns

    out = np.zeros((B, C, S), dtype=np.float32)
    for c in range(NCORES):
        out[c // 4] += res.results[c]["o"]
    return out.reshape(B, C, HH, WW)
